# revision 4
# baseline (speedup 1.0000x reference)
"""NWNet (retrieval-knn) Trainium2 kernel, 8 NeuronCores.

Math: feats = concat(x, sx) @ W_feat; q,s = feats @ proj; scores =
-cdist(q, s); out = log(softmax(scores) @ onehot(sy) + eps).

Device strategy:
  * Host folds the featurizer+projection into one matrix WP = W_feat @
    proj_weight (fp32 GEMM), a 2.5x FLOP reduction on device.
  * Data-parallel over the 8192 support rows (1024 per core); the 128
    queries are replicated. Everything is computed transposed
    ([dim, sample] layouts) so the chain qsT -> scores -> class-bucket
    needs no on-device transposes.
  * qsT = WP.T @ [xT | sxT] in bf16 (fp32 PSUM accumulation).
  * dist^2 assembled in PSUM: q.s matmuls plus two rank-1 fp32 matmuls
    adding -|s|^2/2 and -|q|^2/2; ACT does sqrt then exp with a fixed
    exponent offset K_OFF (softmax max subtraction is unnecessary for
    this distance scale, so no cross-core reduction is needed).
  * Per-class partial sums via one-hot matmul, one PSUM accumulation
    group per bank at a time (start=True clears has_written bank-wide).
  * Host combines: sum partials over cores, Z = per-query total mass,
    out = log(partial/Z + eps).
"""

import numpy as np
import ml_dtypes

import concourse.bacc as bacc
import concourse.mybir as mybir
import concourse.tile as tile
from concourse.bass_utils import run_bass_kernel_spmd

BF16 = mybir.dt.bfloat16
F32 = mybir.dt.float32

B = 128          # queries
S_C = 1024       # support rows per core
FIN = 4096       # input features  (KC chunks of 128)
PD = 1024        # projected dim   (PC chunks)
CPAD = 1024      # classes padded 1000 -> 1024 (CC chunks)
N = B + S_C      # 1152 streamed samples per core
KC = FIN // 128  # 32
PC = PD // 128   # 8
SC = S_C // 128  # 8
CC = CPAD // 128 # 8
NT = 3           # n-tiles per matmul pass
NTW = N // NT    # 384

K_OFF = 47.0     # fixed exp offset: probs = exp(K_OFF - dist)
EPS = 1e-12


def build_bass(reps=1):
    """Build the per-core bass program (same NEFF runs on all 8 cores)."""
    nc = bacc.Bacc("TRN2", target_bir_lowering=False, debug=False, num_devices=8)

    wp_d = nc.dram_tensor("wp", [PC, 128, KC * 128], BF16, kind="ExternalInput")
    rxt_d = nc.dram_tensor("rxt", [128, KC * N], BF16, kind="ExternalInput")
    oh_d = nc.dram_tensor("oh", [128, SC * CPAD], BF16, kind="ExternalInput")
    out_d = nc.dram_tensor("outp", [B, CPAD], F32, kind="ExternalOutput")

    Act = mybir.ActivationFunctionType

    with tile.TileContext(nc) as tc:
        with (
            tc.tile_pool(name="rxt", bufs=1) as p_rxt,
            tc.tile_pool(name="w", bufs=3) as p_w,
            tc.tile_pool(name="qs", bufs=1) as p_qs,
            tc.tile_pool(name="oh", bufs=1) as p_oh,
            tc.tile_pool(name="sq", bufs=2) as p_sq,
            tc.tile_pool(name="nsq", bufs=1) as p_nsq,
            tc.tile_pool(name="dist", bufs=2) as p_dist,
            tc.tile_pool(name="probs", bufs=1) as p_probs,
            tc.tile_pool(name="osb", bufs=1) as p_osb,
            tc.tile_pool(name="ps8", bufs=8, space="PSUM") as p_ps,
        ):
            # ---- resident input loads ----
            rxt_sb = p_rxt.tile([128, KC * N], BF16)
            for g in range(8):  # 4 k-chunks per DMA so compute can start early
                w0 = g * 4 * N
                nc.sync.dma_start(
                    out=rxt_sb[:, w0 : w0 + 4 * N], in_=rxt_d[:, w0 : w0 + 4 * N]
                )
            ones_row = p_nsq.tile([1, 128], BF16, tag="ones_row")
            nc.vector.memset(ones_row[:], 1.0)
            ones_col = p_nsq.tile([128, 1], BF16, tag="ones_col")
            nc.vector.memset(ones_col[:], 1.0)
            koff_sb = p_nsq.tile([128, 1], F32, tag="koff")
            nc.vector.memset(koff_sb[:], K_OFF)

            for _rep in range(reps):
                # ---- phase 1: qsT[m2] = WP[:, m2].T @ rxt  (K=FIN) ----
                qs_sb = p_qs.tile([128, PC * N], BF16)
                for m2 in range(PC):
                    w_sb = p_w.tile([128, KC * 128], BF16, tag="w")
                    nc.sync.dma_start(out=w_sb[:], in_=wp_d[m2])
                    ps = [
                        p_ps.tile([128, 512], F32, tag="bank", name=f"mmps{nt}")
                        for nt in range(NT)
                    ]
                    for kc in range(KC):
                        lhs = w_sb[:, kc * 128 : (kc + 1) * 128]
                        for nt in range(NT):
                            nc.tensor.matmul(
                                ps[nt][:, 0:NTW],
                                lhs,
                                rxt_sb[:, kc * N + nt * NTW : kc * N + (nt + 1) * NTW],
                                start=(kc == 0),
                                stop=(kc == KC - 1),
                            )
                    for nt in range(NT):
                        dst = qs_sb[:, m2 * N + nt * NTW : m2 * N + (nt + 1) * NTW]
                        if nt % 2 == 0:
                            nc.scalar.copy(dst, ps[nt][:, 0:NTW])
                        else:
                            nc.vector.tensor_copy(dst, ps[nt][:, 0:NTW])

                # ---- phase 2: norms: nsq[n] = -0.5 * sum_p qsT[p, n]^2 ----
                nps = [
                    p_ps.tile([1, 512], F32, tag="bank", name=f"nps{nt}")
                    for nt in range(NT)
                ]
                for kc3 in range(PC):
                    sq = p_sq.tile([128, N], BF16, tag="sq")
                    src = qs_sb[:, kc3 * N : (kc3 + 1) * N]
                    nc.vector.tensor_mul(sq[:], src, src)
                    for nt in range(NT):
                        nc.tensor.matmul(
                            nps[nt][0:1, 0:NTW],
                            ones_col[:, 0:1],
                            sq[:, nt * NTW : (nt + 1) * NTW],
                            start=(kc3 == 0),
                            stop=(kc3 == PC - 1),
                        )
                nsq_sb = p_nsq.tile([1, N], F32, tag="nsq")
                nsqc_sb = p_nsq.tile([1, N], BF16, tag="nsqc")
                nsqf_sb = p_nsq.tile([1, N], BF16, tag="nsqf")
                for nt in range(NT):
                    nc.scalar.mul(
                        nsq_sb[0:1, nt * NTW : (nt + 1) * NTW], nps[nt][0:1, 0:NTW], -0.5
                    )
                # split -ssq/2 into bf16 coarse + bf16 residual (exact to ~2^-16)
                nc.scalar.copy(nsqc_sb[0:1, :], nsq_sb[0:1, :])
                nc.vector.tensor_sub(nsqf_sb[0:1, :], nsq_sb[0:1, :], nsqc_sb[0:1, :])

                # ---- phase 3: scores + exp per support chunk ----
                # gt = q.s - ssq/2 - qsq/2 = -dist^2/2
                probs_sb = p_probs.tile([128, SC * B], BF16)
                for sc in range(SC):
                    gt = p_ps.tile([128, B], F32, tag="bank", name="gtps")
                    for kc3 in range(PC):
                        nc.tensor.matmul(
                            gt[:],
                            qs_sb[
                                :,
                                kc3 * N + B + sc * 128 : kc3 * N + B + (sc + 1) * 128,
                            ],
                            qs_sb[:, kc3 * N : kc3 * N + B],
                            start=(kc3 == 0),
                            stop=False,
                        )
                    for part in (nsqc_sb, nsqf_sb):
                        nc.tensor.matmul(
                            gt[:],
                            part[0:1, B + sc * 128 : B + (sc + 1) * 128],
                            ones_row[0:1, :],
                            start=False,
                            stop=False,
                        )
                        nc.tensor.matmul(
                            gt[:],
                            ones_row[0:1, :],
                            part[0:1, 0:B],
                            start=False,
                            stop=(part is nsqf_sb),
                        )
                    nc.vector.tensor_scalar_min(gt[:], gt[:], 0.0)
                    dist = p_dist.tile([128, B], F32, tag="dist")
                    nc.scalar.activation(dist[:], gt[:], Act.Sqrt, bias=0.0, scale=-2.0)
                    nc.scalar.activation(
                        probs_sb[:, sc * B : (sc + 1) * B],
                        dist[:],
                        Act.Exp,
                        bias=koff_sb[:],
                        scale=-1.0,
                    )

                # ---- phase 4: out[b, c] = sum_sc probsT[sc].T @ onehot[sc] ----
                # lhsT = probsT chunk (queries become PSUM partitions), rhs =
                # one-hot rows (classes stream, N=512): 16 matmuls, 2 psum
                # banks, one accumulation group per bank.
                oh_sb = p_oh.tile([128, SC * CPAD], BF16)
                nc.sync.dma_start(out=oh_sb[:], in_=oh_d[:])
                out_sb = p_osb.tile([128, CPAD], F32)
                pos = [
                    p_ps.tile([B, 512], F32, tag="bank", name=f"po{h}")
                    for h in range(2)
                ]
                for sc in range(SC):
                    for h in range(2):
                        nc.tensor.matmul(
                            pos[h][:],
                            probs_sb[:, sc * B : (sc + 1) * B],
                            oh_sb[:, sc * CPAD + h * 512 : sc * CPAD + (h + 1) * 512],
                            start=(sc == 0),
                            stop=(sc == SC - 1),
                        )
                for h in range(2):
                    nc.vector.tensor_copy(
                        out_sb[:, h * 512 : (h + 1) * 512], pos[h][:]
                    )
                nc.sync.dma_start(out=out_d[:], in_=out_sb[:])

    nc.compile()
    return nc


def prep_inputs(x, sx, sy, W_feat, proj_weight):
    """Host-side fold + shard + relayout + bf16 cast; in_maps for 8 cores."""
    bf = ml_dtypes.bfloat16
    x = np.asarray(x, np.float32)
    sx = np.asarray(sx, np.float32)
    sy = np.asarray(sy).astype(np.int64)
    W = np.asarray(W_feat, np.float32)
    P = np.asarray(proj_weight, np.float32)

    # fold featurizer+projection: WP = W @ P  [FIN, PD], slabbed:
    # wp[m2][p][kc*128+m] = WP[kc*128+p, m2*128+m]
    WP = (W @ P).astype(np.float32)
    wp_h = np.ascontiguousarray(
        WP.reshape(KC, 128, PC, 128).transpose(2, 1, 0, 3)
    ).astype(bf).reshape(PC, 128, KC * 128)
    # xT tiles: [p, kc, n] = x[n, kc*128+p]
    xt = np.ascontiguousarray(x.T.reshape(KC, 128, B).transpose(1, 0, 2)).astype(bf)
    # sxT tiles for all cores: [p, kc, i] = sx[i, kc*128+p]
    sxt = np.ascontiguousarray(
        sx.T.reshape(KC, 128, 8 * S_C).transpose(1, 0, 2)
    ).astype(bf)

    in_maps = []
    for c in range(8):
        rxt = np.empty((128, KC, N), bf)
        rxt[:, :, :B] = xt
        rxt[:, :, B:] = sxt[:, :, c * S_C : (c + 1) * S_C]
        sy_c = sy[c * S_C : (c + 1) * S_C]
        oh = np.zeros((S_C, CPAD), np.float32)
        oh[np.arange(S_C), sy_c] = 1.0
        oh_h = np.ascontiguousarray(
            oh.reshape(SC, 128, CPAD).transpose(1, 0, 2)
        ).astype(bf).reshape(128, SC * CPAD)
        in_maps.append(
            {"wp": wp_h, "rxt": rxt.reshape(128, KC * N), "oh": oh_h}
        )
    return in_maps


def combine_outputs(outs):
    """outs: 8 arrays [B, CPAD] f32 -> final [B, 1000] f32."""
    total = np.zeros((B, CPAD), np.float64)
    for o in outs:
        total += o.astype(np.float64)
    Z = total.sum(axis=1)  # padded class columns are exactly zero
    return np.log(total[:, :1000] / Z[:, None] + EPS).astype(np.float32)


_NC_CACHE = {}


def kernel(x, sx, sy, W_feat, proj_weight):
    in_maps = prep_inputs(x, sx, sy, W_feat, proj_weight)
    if "nc" not in _NC_CACHE:
        _NC_CACHE["nc"] = build_bass()
    nc = _NC_CACHE["nc"]
    last_err = None
    for _attempt in range(2):
        try:
            res = run_bass_kernel_spmd(nc, in_maps, list(range(8))).results
            return combine_outputs([res[c]["outp"] for c in range(8)])
        except Exception as e:  # transient device faults: retry once
            last_err = e
            import time as _time

            _time.sleep(2.0)
    raise last_err
